# revision 22
# baseline (speedup 1.0000x reference)
"""Trainium2 Bass kernel for nn_DynamicLogicalNetwork.

Data-parallel over batch: B=8 batch elements -> 8 NeuronCores, weights
replicated, no collectives.  Returns (logits [8,2048,64], attn [8,2048,2048]).
"""

import sys

for _p in ("/opt/trn_rl_repo",):
    if _p not in sys.path:
        sys.path.insert(0, _p)

import numpy as np

# Model dims (hardcoded per problem spec)
B, S, DIN = 8, 2048, 1024
H1, H2, C = 2048, 1024, 64
LN_EPS = 1e-5
SCALE = 1.0 / np.sqrt(np.float32(H2))  # 1/32

SB = 512          # s-block size in phase 1
NSB = S // SB     # 4
KD = DIN // 128   # 8  k-tiles over D_IN
MH1 = H1 // 128   # 16 m-tiles over H1
ND2 = H2 // 512   # 2  d chunks of 512
MT = S // 128     # 16 row blocks
NT = S // 512     # 4  col blocks
KH2 = H2 // 128   # 8  k-tiles over H2

_BUILT = {}


def _build():
    import concourse.bass as bass
    import concourse.bacc as bacc
    import concourse.mybir as mybir
    import concourse.tile as tile

    f32 = mybir.dt.float32
    f32r = mybir.dt.float32r
    AF = mybir.ActivationFunctionType
    ALU = mybir.AluOpType

    nc = bacc.Bacc("TRN2", target_bir_lowering=False, debug=False, num_devices=8)

    def din(name, shape, dt=f32):
        return nc.dram_tensor(name, shape, dt, kind="ExternalInput").ap()

    x_d = din("x", [S, DIN], f32r)
    prior_d = din("prior", [C])
    w1_d = din("w1", [DIN, H1], f32r)
    b1_d = din("b1", [H1])
    g1_d = din("g1", [H1])
    be1_d = din("be1", [H1])
    w2_d = din("w2", [H1, H2], f32r)
    b2_d = din("b2", [H2])
    g2_d = din("g2", [H2])
    be2_d = din("be2", [H2])
    wh_d = din("wh", [H2, 128], f32r)
    bh_d = din("bh", [C])
    ag_d = din("ag", [C])
    og_d = din("og", [C])
    eye_d = din("eye", [128, 128])
    eyer_d = din("eyer", [128, 128], f32r)
    ones_d = din("ones", [128, 128], f32r)

    attn_d = nc.dram_tensor("attn_out", [S, S], f32, kind="ExternalOutput").ap()
    logit_d = nc.dram_tensor("logits_out", [S, C], f32, kind="ExternalOutput").ap()

    f_d = nc.dram_tensor("f_scr", [S, H2], f32r).ap()
    fT_d = nc.dram_tensor("fT_scr", [H2, S], f32r).ap()
    e_d = nc.dram_tensor("e_scr", [S, S], f32r).ap()
    zi_d = nc.dram_tensor("zi_scr", [S], f32).ap()

    def mmr(out, lhsT, rhs, start, stop):
        nc.tensor.matmul(out, lhsT, rhs, start=start, stop=stop)

    with tile.TileContext(nc) as tc:
        cp = tc.alloc_tile_pool(name="const", bufs=1)
        eye = cp.tile([128, 128], f32)
        eyer = cp.tile([128, 128], f32r)
        nc.sync.dma_start(eyer[:], eyer_d[:])
        nc.sync.dma_start(eye[:], eye_d[:])
        ones = cp.tile([128, 128], f32r)
        nc.sync.dma_start(ones[:], ones_d[:])
        b1c = cp.tile([128, MH1], f32)
        nc.sync.dma_start(b1c[:], b1_d.rearrange("(t p) -> p t", p=128))
        g1c = cp.tile([128, MH1], f32)
        nc.sync.dma_start(g1c[:], g1_d.rearrange("(t p) -> p t", p=128))
        be1c = cp.tile([128, MH1], f32)
        nc.sync.dma_start(be1c[:], be1_d.rearrange("(t p) -> p t", p=128))
        b2row = cp.tile([1, H2], f32)
        nc.sync.dma_start(b2row[:], b2_d.rearrange("(o d) -> o d", o=1))
        g2bc = cp.tile([128, H2], f32)
        nc.sync.dma_start(
            g2bc[:], g2_d.rearrange("(o d) -> o d", o=1).to_broadcast((128, H2)))
        be2bc = cp.tile([128, H2], f32)
        nc.sync.dma_start(
            be2bc[:], be2_d.rearrange("(o d) -> o d", o=1).to_broadcast((128, H2)))
        whsb = cp.tile([128, KH2, 128], f32r)
        nc.sync.dma_start(whsb[:], wh_d.rearrange("(k p) c -> p k c", p=128))
        bhc = cp.tile([C, 1], f32)
        nc.sync.dma_start(bhc[:], bh_d.rearrange("(c o) -> c o", o=1))
        agc = cp.tile([C, 1], f32)
        nc.sync.dma_start(agc[:], ag_d.rearrange("(c o) -> c o", o=1))
        ogc = cp.tile([C, 1], f32)
        nc.sync.dma_start(ogc[:], og_d.rearrange("(c o) -> c o", o=1))
        prc = cp.tile([C, 1], f32)
        nc.sync.dma_start(prc[:], prior_d.rearrange("(c o) -> c o", o=1))
        urow = cp.tile([1, H2], f32r)
        vrow = cp.tile([1, H2], f32r)
        epsc = cp.tile([128, 1], f32)
        nc.gpsimd.memset(epsc[:], LN_EPS)
        e10c = cp.tile([128, 1], f32)
        nc.gpsimd.memset(e10c[:], 1e-10)

        # ---------------- Phase 1: MLP  x -> f (and fT) ----------------
        with tc.tile_pool(name="w2p", bufs=1) as w2p, \
             tc.tile_pool(name="xtp", bufs=1) as xtp, \
             tc.tile_pool(name="w1p", bufs=3) as w1p, \
             tc.tile_pool(name="htp", bufs=1) as htp, \
             tc.tile_pool(name="sqp", bufs=2) as sqp, \
             tc.tile_pool(name="fwp", bufs=2) as fwp, \
             tc.tile_pool(name="fop", bufs=3) as fop, \
             tc.tile_pool(name="ftp", bufs=2) as ftp, \
             tc.tile_pool(name="strow", bufs=1) as strow, \
             tc.tile_pool(name="stcol", bufs=2) as stcol, \
             tc.tile_pool(name="psA", bufs=2, space="PSUM") as psA, \
             tc.tile_pool(name="psSt", bufs=2, space="PSUM") as psSt, \
             tc.tile_pool(name="psB", bufs=2, space="PSUM") as psB, \
             tc.tile_pool(name="psT", bufs=2, space="PSUM") as psT:

            w2sb = w2p.tile([128, MH1, H2], f32r)
            nc.sync.dma_start(w2sb[:], w2_d.rearrange("(t p) d -> p t d", p=128))

            # v = beta1 @ W2 + b2  (before the in-place g1 scaling of W2)
            for dc in range(ND2):
                pv = psSt.tile([128, 512], f32, tag="pstat")
                for jt in range(MH1):
                    vt = sqp.tile([128, 512], f32r, tag="vtmp")
                    nc.vector.tensor_scalar_mul(
                        vt[:], w2sb[:, jt, dc * 512:(dc + 1) * 512],
                        be1c[:, jt:jt + 1])
                    mmr(pv[:], ones[:], vt[:],
                        start=(jt == 0), stop=(jt == MH1 - 1))
                nc.vector.tensor_add(vrow[:, dc * 512:(dc + 1) * 512], pv[0:1, :],
                                     b2row[:, dc * 512:(dc + 1) * 512])
            # W2 <- g1 * W2 (in place), then u = colsum(W2g)
            for jt in range(MH1):
                nc.vector.tensor_scalar_mul(w2sb[:, jt, :], w2sb[:, jt, :],
                                            g1c[:, jt:jt + 1])
            for dc in range(ND2):
                pu = psSt.tile([128, 512], f32, tag="pstat")
                for jt in range(MH1):
                    mmr(pu[:], ones[:], w2sb[:, jt, dc * 512:(dc + 1) * 512],
                        start=(jt == 0), stop=(jt == MH1 - 1))
                nc.scalar.copy(urow[:, dc * 512:(dc + 1) * 512], pu[0:1, :])

            for sb in range(NSB):
                s0 = sb * SB
                xt = xtp.tile([128, KD, SB], f32r)
                for k in range(KD):
                    nc.sync.dma_start(
                        xt[:, k, :],
                        x_d[s0:s0 + SB, k * 128:(k + 1) * 128]
                        .rearrange("s p -> p s"))

                hT = htp.tile([128, MH1, SB], f32r)
                psh = psSt.tile([128, SB], f32, tag="pstat")
                psq = psSt.tile([128, SB], f32, tag="pstat")
                # (both live across the m loop; pool bufs=2 gives exactly 2 slots)
                for m in range(MH1):
                    w1t = w1p.tile([128, KD, 128], f32r)
                    nc.sync.dma_start(
                        w1t[:], w1_d[:, m * 128:(m + 1) * 128]
                        .rearrange("(k p) m -> p k m", p=128))
                    pa = psA.tile([128, SB], f32)
                    for k in range(KD):
                        mmr(pa[:], w1t[:, k, :], xt[:, k, :],
                            start=(k == 0), stop=(k == KD - 1))
                    nc.scalar.activation(hT[:, m, :], pa[:], AF.Relu,
                                         bias=b1c[:, m:m + 1])
                    mmr(psh[:], ones[:], hT[:, m, :],
                        start=(m == 0), stop=(m == MH1 - 1))
                    sq = sqp.tile([128, SB], f32r)
                    nc.scalar.activation(sq[:], hT[:, m, :], AF.Square)
                    mmr(psq[:], ones[:], sq[:],
                        start=(m == 0), stop=(m == MH1 - 1))

                # LN1 stats (rows, on partition 0)
                negmu = strow.tile([1, SB], f32r, tag="negmu")
                nc.scalar.activation(negmu[:], psh[0:1, :], AF.Copy, scale=-1.0 / H1)
                mu2r = strow.tile([1, SB], f32, tag="mu2r")
                nc.scalar.activation(mu2r[:], psh[0:1, :], AF.Square, scale=1.0 / H1)
                s2r = strow.tile([1, SB], f32, tag="s2r")
                nc.scalar.activation(s2r[:], psq[0:1, :], AF.Copy, scale=1.0 / H1)
                varr = strow.tile([1, SB], f32, tag="varr")
                nc.vector.tensor_sub(varr[:], s2r[:], mu2r[:])
                rinv = strow.tile([1, SB], f32r, tag="rinv")
                nc.scalar.activation(rinv[:], varr[:], AF.Sqrt, bias=epsc[:1])

                # r as columns per 128-s-chunk (PE transpose of rinv row)
                rcol = stcol.tile([128, SB // 128], f32, tag="rcol")
                for sc in range(SB // 128):
                    pt = psT.tile([128, 2], f32, tag="pt")
                    mmr(pt[:], rinv[:, sc * 128:(sc + 1) * 128], ones[0:1, 0:2],
                        start=True, stop=True)
                    nc.vector.reciprocal(rcol[:, sc:sc + 1], pt[:, 0:1])

                # Stage B per (s-chunk 128, d-chunk 512)
                for sc in range(SB // 128):
                    relu2 = fwp.tile([128, H2], f32)
                    s1a = stcol.tile([128, 1], f32, tag="s1a")
                    s1b = stcol.tile([128, 1], f32, tag="s1b")
                    s2a = stcol.tile([128, 1], f32, tag="s2a")
                    s2b = stcol.tile([128, 1], f32, tag="s2b")
                    for dc in range(ND2):
                        pb = psB.tile([128, 512], f32)
                        for jt in range(MH1):
                            mmr(pb[:], hT[:, jt, sc * 128:(sc + 1) * 128],
                                w2sb[:, jt, dc * 512:(dc + 1) * 512],
                                start=(jt == 0), stop=False)
                        mmr(pb[:], negmu[:, sc * 128:(sc + 1) * 128],
                            urow[:, dc * 512:(dc + 1) * 512], start=False, stop=False)
                        mmr(pb[:], rinv[:, sc * 128:(sc + 1) * 128],
                            vrow[:, dc * 512:(dc + 1) * 512], start=False, stop=True)
                        nc.scalar.activation(
                            relu2[:, dc * 512:(dc + 1) * 512], pb[:], AF.Relu,
                            accum_out=(s1a if dc == 0 else s1b)[:])
                        sq2 = sqp.tile([128, 512], f32, tag="sq2")
                        nc.scalar.activation(
                            sq2[:], relu2[:, dc * 512:(dc + 1) * 512], AF.Square,
                            accum_out=(s2a if dc == 0 else s2b)[:])
                    s1 = stcol.tile([128, 1], f32, tag="s1")
                    nc.vector.tensor_add(s1[:], s1a[:], s1b[:])
                    s2 = stcol.tile([128, 1], f32, tag="s2")
                    nc.vector.tensor_add(s2[:], s2a[:], s2b[:])
                    # mu2 = r*s1/H2 ; e2 = r^2*s2/H2 ; var2 = e2 - mu2^2
                    rc = rcol[:, sc:sc + 1]
                    mu2 = stcol.tile([128, 1], f32, tag="mu2")
                    nc.vector.tensor_scalar(mu2[:], s1[:], rc, 1.0 / H2,
                                            ALU.mult, ALU.mult)
                    e2 = stcol.tile([128, 1], f32, tag="e2")
                    nc.vector.tensor_scalar(e2[:], s2[:], rc, 1.0 / H2,
                                            ALU.mult, ALU.mult)
                    nc.vector.tensor_scalar(e2[:], e2[:], rc, None, ALU.mult)
                    mu2sq = stcol.tile([128, 1], f32, tag="mu2sq")
                    nc.scalar.activation(mu2sq[:], mu2[:], AF.Square)
                    var2 = stcol.tile([128, 1], f32, tag="var2")
                    nc.vector.tensor_sub(var2[:], e2[:], mu2sq[:])
                    sd2 = stcol.tile([128, 1], f32, tag="sd2")
                    nc.scalar.activation(sd2[:], var2[:], AF.Sqrt, bias=epsc[:])
                    r2 = stcol.tile([128, 1], f32, tag="r2")
                    nc.vector.reciprocal(r2[:], sd2[:])
                    c1 = stcol.tile([128, 1], f32, tag="c1")
                    nc.vector.tensor_mul(c1[:], rc, r2[:])
                    c0 = stcol.tile([128, 1], f32, tag="c0")
                    nc.vector.tensor_scalar(c0[:], mu2[:], r2[:], -1.0,
                                            ALU.mult, ALU.mult)

                    fTst = ftp.tile([128, KH2, 128], f32r)
                    for dc in range(ND2):
                        fnat = fop.tile([128, 512], f32)
                        nc.vector.tensor_scalar(fnat[:], relu2[:, dc * 512:(dc + 1) * 512],
                                                c1[:], c0[:], ALU.mult, ALU.add)
                        nc.vector.tensor_mul(fnat[:], fnat[:],
                                             g2bc[:, dc * 512:(dc + 1) * 512])
                        fnr = fop.tile([128, 512], f32r, tag="fnr")
                        nc.vector.tensor_add(fnr[:], fnat[:],
                                             be2bc[:, dc * 512:(dc + 1) * 512])
                        nc.sync.dma_start(
                            f_d[s0 + sc * 128:s0 + (sc + 1) * 128,
                                dc * 512:(dc + 1) * 512], fnr[:])
                        for q in range(4):
                            ptr = psT.tile([128, 128], f32r, tag="pt")
                            nc.tensor.transpose(ptr[:], fnr[:, q * 128:(q + 1) * 128],
                                                eyer[:])
                            nc.scalar.copy(fTst[:, dc * 4 + q, :], ptr[:])
                    nc.sync.dma_start(
                        fT_d[:, s0 + sc * 128:s0 + (sc + 1) * 128]
                        .rearrange("(k p) s -> p k s", p=128), fTst[:])

        # ---------------- Phase 2a: scores, softmax, E, attn ----------------
        zic = None
        with tc.tile_pool(name="ftr", bufs=1) as ftr, \
             tc.tile_pool(name="ep", bufs=2) as ep, \
             tc.tile_pool(name="ap", bufs=2) as ap_, \
             tc.tile_pool(name="zp", bufs=1) as zp, \
             tc.tile_pool(name="psR", bufs=2, space="PSUM") as psR:
            fTres = ftr.tile([128, KH2, S], f32r)
            nc.sync.dma_start(fTres[:], fT_d.rearrange("(k p) t -> p k t", p=128))
            ziall = zp.tile([128, MT], f32)
            for a in range(MT):
                praw = psR.tile([128, S], f32)
                for n in range(NT):
                    for dt in range(KH2):
                        mmr(praw[:, n * 512:(n + 1) * 512],
                            fTres[:, dt, a * 128:(a + 1) * 128],
                            fTres[:, dt, n * 512:(n + 1) * 512],
                            start=(dt == 0), stop=(dt == KH2 - 1))
                et = ep.tile([128, S], f32r)
                zcol = zp.tile([128, 1], f32, tag="zcol")
                nc.scalar.activation(et[:], praw[:], AF.Exp, scale=float(SCALE),
                                     accum_out=zcol[:])
                nc.vector.reciprocal(ziall[:, a:a + 1], zcol[:])
                at = ap_.tile([128, S], f32)
                nc.vector.tensor_scalar_mul(at[:], et[:], ziall[:, a:a + 1])
                nc.sync.dma_start(attn_d[a * 128:(a + 1) * 128, :], at[:])
                nc.sync.dma_start(e_d[a * 128:(a + 1) * 128, :], et[:])
            nc.sync.dma_start(zi_d.rearrange("(a p) -> p a", p=128), ziall[:])

        # ---------------- Phase 2b: attended, head, epilogue ----------------
        with tc.tile_pool(name="fr", bufs=1) as fr, \
             tc.tile_pool(name="esp", bufs=6) as esp, \
             tc.tile_pool(name="atp", bufs=2) as atp, \
             tc.tile_pool(name="lgp", bufs=1) as lgp, \
             tc.tile_pool(name="lsp", bufs=1) as lsp, \
             tc.tile_pool(name="psAT", bufs=8, space="PSUM") as psAT:
            fres = fr.tile([128, MT, H2], f32r)
            nc.sync.dma_start(fres[:], f_d.rearrange("(t p) d -> p t d", p=128))
            zbc = lgp.tile([C, S], f32)
            nc.sync.dma_start(
                zbc[:], zi_d.rearrange("(o s) -> o s", o=1).to_broadcast((C, S)))
            logT = lgp.tile([C, S], f32)

            for n in range(NT):
                attT = atp.tile([128, KH2, 512], f32r)
                pms = [psAT.tile([128, 512], f32, tag="acc", name=f"pm{m}")
                       for m in range(KH2)]
                for tt in range(MT):
                    et = esp.tile([128, 512], f32r, tag="et")
                    nc.sync.dma_start(
                        et[:], e_d[tt * 128:(tt + 1) * 128,
                                   n * 512:(n + 1) * 512])
                    for m in range(KH2):
                        mmr(pms[m][:], fres[:, tt, m * 128:(m + 1) * 128],
                            et[:], start=(tt == 0), stop=(tt == MT - 1))
                for m in range(KH2):
                    nc.scalar.copy(attT[:, m, :], pms[m][:])
                ph = psAT.tile([128, 512], f32, tag="acc")
                for k in range(KH2):
                    mmr(ph[:], whsb[:, k, :], attT[:, k, :],
                        start=(k == 0), stop=(k == KH2 - 1))
                nc.scalar.copy(logT[:, n * 512:(n + 1) * 512], ph[0:C, :])

            # epilogue on [64, 2048]
            nc.vector.tensor_mul(logT[:], logT[:], zbc[:])
            sigl = lgp.tile([C, S], f32)
            nc.scalar.activation(sigl[:], logT[:], AF.Sigmoid, bias=bhc[:])
            sigp = lgp.tile([C, 1], f32)
            nc.scalar.activation(sigp[:], prc[:], AF.Sigmoid)
            andv = lgp.tile([C, S], f32)
            nc.vector.tensor_scalar(andv[:], sigl[:], sigp[:], agc[:],
                                    ALU.min, ALU.mult)
            orv = lgp.tile([C, S], f32)
            nc.vector.tensor_scalar(orv[:], sigl[:], sigp[:], ogc[:],
                                    ALU.max, ALU.mult)
            nc.vector.tensor_add(andv[:], andv[:], orv[:])
            nc.scalar.activation(logT[:], andv[:], AF.Ln, bias=e10c[:C], scale=0.5)

            lst = lsp.tile([128, MT, C], f32)
            for q in range(MT):
                pl = psAT.tile([128, C], f32, tag="acc")
                nc.tensor.transpose(pl[:], logT[:, q * 128:(q + 1) * 128],
                                    eye[:C, :C])
                nc.scalar.copy(lst[:, q, :], pl[:])
            nc.sync.dma_start(logit_d.rearrange("(q p) c -> p q c", p=128), lst[:])

        cp.release()

    nc.compile()
    return nc


def _get_nc():
    if "nc" not in _BUILT:
        _BUILT["nc"] = _build()
    return _BUILT["nc"]


def kernel(**inputs):
    from concourse.bass_utils import run_bass_kernel_spmd

    nc = _get_nc()
    emb = np.ascontiguousarray(np.asarray(inputs["embeddings"], dtype=np.float32))
    prior = np.ascontiguousarray(np.asarray(inputs["prior_evidence"], dtype=np.float32))
    common = {
        "w1": np.asarray(inputs["W1"], np.float32),
        "b1": np.asarray(inputs["b1"], np.float32),
        "g1": np.asarray(inputs["g1"], np.float32),
        "be1": np.asarray(inputs["beta1"], np.float32),
        "w2": np.asarray(inputs["W2"], np.float32),
        "b2": np.asarray(inputs["b2"], np.float32),
        "g2": np.asarray(inputs["g2"], np.float32),
        "be2": np.asarray(inputs["beta2"], np.float32),
        "wh": np.pad(np.asarray(inputs["Wh"], np.float32), ((0, 0), (0, 64))),
        "bh": np.asarray(inputs["bh"], np.float32),
        "ag": np.asarray(inputs["and_gate"], np.float32),
        "og": np.asarray(inputs["or_gate"], np.float32),
        "eye": np.eye(128, dtype=np.float32),
        "eyer": np.eye(128, dtype=np.float32),
        "ones": np.ones((128, 128), np.float32),
    }
    common = {k: np.ascontiguousarray(v) for k, v in common.items()}
    in_maps = [dict(common, x=emb[b], prior=prior[b]) for b in range(B)]
    res = run_bass_kernel_spmd(nc, in_maps, list(range(B)))
    logits = np.stack([res.results[b]["logits_out"] for b in range(B)])
    attn = np.stack([res.results[b]["attn_out"] for b in range(B)])
    return (logits, attn)


if __name__ == "__main__":
    _get_nc()
    print("build+compile OK")


# revision 25
# speedup vs baseline: 2.3409x; 2.3409x over previous
"""Trainium2 Bass kernel for nn_DynamicLogicalNetwork.

Data-parallel over batch: B=8 batch elements -> 8 NeuronCores, weights
replicated, no collectives.  Returns (logits [8,2048,64], attn [8,2048,2048]).
"""

import sys

for _p in ("/opt/trn_rl_repo",):
    if _p not in sys.path:
        sys.path.insert(0, _p)

import numpy as np

# Model dims (hardcoded per problem spec)
B, S, DIN = 8, 2048, 1024
H1, H2, C = 2048, 1024, 64
LN_EPS = 1e-5
SCALE = 1.0 / np.sqrt(np.float32(H2))  # 1/32

SB = 512          # s-block size in phase 1
NSB = S // SB     # 4
KD = DIN // 128   # 8  k-tiles over D_IN
MH1 = H1 // 128   # 16 m-tiles over H1
ND2 = H2 // 512   # 2  d chunks of 512
MT = S // 128     # 16 row blocks
NT = S // 512     # 4  col blocks
KH2 = H2 // 128   # 8  k-tiles over H2

_BUILT = {}


def _build():
    import concourse.bass as bass
    import concourse.bacc as bacc
    import concourse.mybir as mybir
    import concourse.tile as tile

    f32 = mybir.dt.float32
    f32r = mybir.dt.float32r
    AF = mybir.ActivationFunctionType
    ALU = mybir.AluOpType

    nc = bacc.Bacc("TRN2", target_bir_lowering=False, debug=False, num_devices=8)

    def din(name, shape, dt=f32):
        return nc.dram_tensor(name, shape, dt, kind="ExternalInput").ap()

    x_d = din("x", [S, DIN], f32r)
    prior_d = din("prior", [C])
    w1_d = din("w1", [DIN, H1], f32r)
    b1_d = din("b1", [H1])
    g1_d = din("g1", [H1])
    be1_d = din("be1", [H1])
    w2_d = din("w2", [H1, H2], f32r)
    b2_d = din("b2", [H2])
    g2_d = din("g2", [H2])
    be2_d = din("be2", [H2])
    wh_d = din("wh", [H2, 128], f32r)
    bh_d = din("bh", [C])
    ag_d = din("ag", [C])
    og_d = din("og", [C])
    eye_d = din("eye", [128, 128])
    eyer_d = din("eyer", [128, 128], f32r)
    ones_d = din("ones", [128, 128], f32r)

    attn_d = nc.dram_tensor("attn_out", [S, S], f32, kind="ExternalOutput").ap()
    logit_d = nc.dram_tensor("logits_out", [S, C], f32, kind="ExternalOutput").ap()

    f_d = nc.dram_tensor("f_scr", [S, H2], f32r).ap()
    fT_d = nc.dram_tensor("fT_scr", [H2, S], f32r).ap()
    e_d = nc.dram_tensor("e_scr", [S, S], f32r).ap()
    zi_d = nc.dram_tensor("zi_scr", [S], f32).ap()

    def mmr(out, lhsT, rhs, start, stop):
        nc.tensor.matmul(out, lhsT, rhs, start=start, stop=stop)

    with tile.TileContext(nc) as tc:
        cp = tc.alloc_tile_pool(name="const", bufs=1)
        eye = cp.tile([128, 128], f32)
        eyer = cp.tile([128, 128], f32r)
        nc.sync.dma_start(eyer[:], eyer_d[:])
        nc.sync.dma_start(eye[:], eye_d[:])
        ones = cp.tile([128, 128], f32r)
        nc.sync.dma_start(ones[:], ones_d[:])
        b1c = cp.tile([128, MH1], f32)
        nc.sync.dma_start(b1c[:], b1_d.rearrange("(t p) -> p t", p=128))
        g1c = cp.tile([128, MH1], f32)
        nc.sync.dma_start(g1c[:], g1_d.rearrange("(t p) -> p t", p=128))
        be1c = cp.tile([128, MH1], f32)
        nc.sync.dma_start(be1c[:], be1_d.rearrange("(t p) -> p t", p=128))
        b2row = cp.tile([1, H2], f32)
        nc.sync.dma_start(b2row[:], b2_d.rearrange("(o d) -> o d", o=1))
        g2bc = cp.tile([128, H2], f32)
        nc.sync.dma_start(
            g2bc[:], g2_d.rearrange("(o d) -> o d", o=1).to_broadcast((128, H2)))
        be2bc = cp.tile([128, H2], f32)
        nc.sync.dma_start(
            be2bc[:], be2_d.rearrange("(o d) -> o d", o=1).to_broadcast((128, H2)))
        whsb = cp.tile([128, KH2, 128], f32r)
        nc.sync.dma_start(whsb[:], wh_d.rearrange("(k p) c -> p k c", p=128))
        bhc = cp.tile([C, 1], f32)
        nc.sync.dma_start(bhc[:], bh_d.rearrange("(c o) -> c o", o=1))
        agc = cp.tile([C, 1], f32)
        nc.sync.dma_start(agc[:], ag_d.rearrange("(c o) -> c o", o=1))
        ogc = cp.tile([C, 1], f32)
        nc.sync.dma_start(ogc[:], og_d.rearrange("(c o) -> c o", o=1))
        prc = cp.tile([C, 1], f32)
        nc.sync.dma_start(prc[:], prior_d.rearrange("(c o) -> c o", o=1))
        uvrow = cp.tile([1, 2 * H2], f32r)
        urow = uvrow[:, 0:H2]
        vrow = uvrow[:, H2:2 * H2]
        epsc = cp.tile([128, 1], f32)
        nc.gpsimd.memset(epsc[:], LN_EPS)
        e10c = cp.tile([128, 1], f32)
        nc.gpsimd.memset(e10c[:], 1e-10)

        # ---------------- Phase 1: MLP  x -> f (and fT) ----------------
        with tc.tile_pool(name="w2p", bufs=1) as w2p, \
             tc.tile_pool(name="xtp", bufs=1) as xtp, \
             tc.tile_pool(name="w1p", bufs=2) as w1p, \
             tc.tile_pool(name="htp", bufs=1) as htp, \
             tc.tile_pool(name="sqp", bufs=1) as sqp, \
             tc.tile_pool(name="fwp", bufs=1) as fwp, \
             tc.tile_pool(name="fop", bufs=2) as fop, \
             tc.tile_pool(name="ftp", bufs=1) as ftp, \
             tc.tile_pool(name="strow", bufs=1) as strow, \
             tc.tile_pool(name="stcol", bufs=2) as stcol, \
             tc.tile_pool(name="psA", bufs=2, space="PSUM") as psA, \
             tc.tile_pool(name="psSt", bufs=2, space="PSUM") as psSt, \
             tc.tile_pool(name="psB", bufs=2, space="PSUM") as psB, \
             tc.tile_pool(name="psT", bufs=2, space="PSUM") as psT:

            w2sb = w2p.tile([128, MH1, H2], f32r)
            nc.sync.dma_start(w2sb[:], w2_d.rearrange("(t p) d -> p t d", p=128))

            # v = beta1 @ W2 + b2  (before the in-place g1 scaling of W2)
            for dc in range(ND2):
                pv = psSt.tile([128, 512], f32, tag="pstat")
                for jt in range(MH1):
                    vt = sqp.tile([128, 512], f32r, tag="vtmp")
                    nc.vector.tensor_scalar_mul(
                        vt[:], w2sb[:, jt, dc * 512:(dc + 1) * 512],
                        be1c[:, jt:jt + 1])
                    mmr(pv[:], ones[:], vt[:],
                        start=(jt == 0), stop=(jt == MH1 - 1))
                nc.vector.tensor_add(vrow[:, dc * 512:(dc + 1) * 512], pv[0:1, :],
                                     b2row[:, dc * 512:(dc + 1) * 512])
            # W2 <- g1 * W2 (in place), then u = colsum(W2g)
            for jt in range(MH1):
                nc.vector.tensor_scalar_mul(w2sb[:, jt, :], w2sb[:, jt, :],
                                            g1c[:, jt:jt + 1])
            for dc in range(ND2):
                pu = psSt.tile([128, 512], f32, tag="pstat")
                for jt in range(MH1):
                    mmr(pu[:], ones[:], w2sb[:, jt, dc * 512:(dc + 1) * 512],
                        start=(jt == 0), stop=(jt == MH1 - 1))
                nc.scalar.copy(urow[:, dc * 512:(dc + 1) * 512], pu[0:1, :])

            for sb in range(NSB):
                s0 = sb * SB
                xt = xtp.tile([128, KD, SB], f32r)
                for sc in range(SB // 128):
                    xn = xtp.tile([128, DIN], f32r, tag="xn", bufs=2)
                    nc.sync.dma_start(
                        xn[:], x_d[s0 + sc * 128:s0 + (sc + 1) * 128, :])
                    for k in range(KD):
                        pxt = psT.tile([128, 128], f32r, tag="pt", name="pxt")
                        nc.tensor.transpose(
                            pxt[:], xn[:, k * 128:(k + 1) * 128], eyer[:])
                        nc.scalar.copy(xt[:, k, sc * 128:(sc + 1) * 128], pxt[:])

                hT = htp.tile([128, MH1, SB], f32r)
                psh = psSt.tile([128, SB], f32, tag="pstat")
                psq = psSt.tile([128, SB], f32, tag="pstat")
                # (both live across the m loop; pool bufs=2 gives exactly 2 slots)
                for m in range(MH1):
                    w1t = w1p.tile([128, KD, 128], f32r)
                    nc.sync.dma_start(
                        w1t[:], w1_d[:, m * 128:(m + 1) * 128]
                        .rearrange("(k p) m -> p k m", p=128))
                    pa = psA.tile([128, SB], f32)
                    for k in range(KD):
                        mmr(pa[:], w1t[:, k, :], xt[:, k, :],
                            start=(k == 0), stop=(k == KD - 1))
                    nc.scalar.activation(hT[:, m, :], pa[:], AF.Relu,
                                         bias=b1c[:, m:m + 1])
                    mmr(psh[:], ones[:], hT[:, m, :],
                        start=(m == 0), stop=(m == MH1 - 1))
                    sq = sqp.tile([128, SB], f32r)
                    nc.scalar.activation(sq[:], hT[:, m, :], AF.Square)
                    mmr(psq[:], ones[:], sq[:],
                        start=(m == 0), stop=(m == MH1 - 1))

                # LN1 stats (rows, on partition 0)
                negmu = strow.tile([1, SB], f32r, tag="negmu")
                nc.scalar.activation(negmu[:], psh[0:1, :], AF.Copy, scale=-1.0 / H1)
                mu2r = strow.tile([1, SB], f32, tag="tmp", bufs=3)
                nc.scalar.activation(mu2r[:], psh[0:1, :], AF.Square, scale=1.0 / H1)
                s2r = strow.tile([1, SB], f32, tag="tmp", bufs=3)
                nc.scalar.activation(s2r[:], psq[0:1, :], AF.Copy, scale=1.0 / H1)
                varr = strow.tile([1, SB], f32, tag="tmp", bufs=3)
                nc.vector.tensor_sub(varr[:], s2r[:], mu2r[:])
                rinv = strow.tile([1, SB], f32r, tag="rinv")
                nc.scalar.activation(rinv[:], varr[:], AF.Sqrt, bias=epsc[:1])

                # r as columns per 128-s-chunk (PE transpose of rinv row)
                rcol = stcol.tile([128, SB // 128], f32, tag="rcol")
                for sc in range(SB // 128):
                    pt = psT.tile([128, 2], f32, tag="pt")
                    mmr(pt[:], rinv[:, sc * 128:(sc + 1) * 128], ones[0:1, 0:2],
                        start=True, stop=True)
                    nc.vector.reciprocal(rcol[:, sc:sc + 1], pt[:, 0:1])

                # Stage B per (s-chunk 128, d-chunk 512)
                fTacc = ftp.tile([128, KH2, SB], f32r)
                for sc in range(SB // 128):
                    relu2 = fwp.tile([128, H2], f32)
                    s1a = stcol.tile([128, 1], f32, tag="s1a")
                    s1b = stcol.tile([128, 1], f32, tag="s1b")
                    s2a = stcol.tile([128, 1], f32, tag="s2a")
                    s2b = stcol.tile([128, 1], f32, tag="s2b")
                    for dc in range(ND2):
                        pb = psB.tile([128, 512], f32)
                        for jt in range(MH1):
                            mmr(pb[:], hT[:, jt, sc * 128:(sc + 1) * 128],
                                w2sb[:, jt, dc * 512:(dc + 1) * 512],
                                start=(jt == 0), stop=False)
                        mmr(pb[:], negmu[:, sc * 128:(sc + 1) * 128],
                            urow[:, dc * 512:(dc + 1) * 512], start=False, stop=False)
                        mmr(pb[:], rinv[:, sc * 128:(sc + 1) * 128],
                            vrow[:, dc * 512:(dc + 1) * 512], start=False, stop=True)
                        nc.scalar.activation(
                            relu2[:, dc * 512:(dc + 1) * 512], pb[:], AF.Relu,
                            accum_out=(s1a if dc == 0 else s1b)[:])
                        sq2 = sqp.tile([128, 512], f32, tag="sq2")
                        nc.scalar.activation(
                            sq2[:], relu2[:, dc * 512:(dc + 1) * 512], AF.Square,
                            accum_out=(s2a if dc == 0 else s2b)[:])
                    s1 = stcol.tile([128, 1], f32, tag="s1")
                    nc.vector.tensor_add(s1[:], s1a[:], s1b[:])
                    s2 = stcol.tile([128, 1], f32, tag="s2")
                    nc.vector.tensor_add(s2[:], s2a[:], s2b[:])
                    # mu2 = r*s1/H2 ; e2 = r^2*s2/H2 ; var2 = e2 - mu2^2
                    rc = rcol[:, sc:sc + 1]
                    mu2 = stcol.tile([128, 1], f32, tag="mu2")
                    nc.vector.tensor_scalar(mu2[:], s1[:], rc, 1.0 / H2,
                                            ALU.mult, ALU.mult)
                    e2 = stcol.tile([128, 1], f32, tag="e2")
                    nc.vector.tensor_scalar(e2[:], s2[:], rc, 1.0 / H2,
                                            ALU.mult, ALU.mult)
                    nc.vector.tensor_scalar(e2[:], e2[:], rc, None, ALU.mult)
                    mu2sq = stcol.tile([128, 1], f32, tag="mu2sq")
                    nc.scalar.activation(mu2sq[:], mu2[:], AF.Square)
                    var2 = stcol.tile([128, 1], f32, tag="var2")
                    nc.vector.tensor_sub(var2[:], e2[:], mu2sq[:])
                    sd2 = stcol.tile([128, 1], f32, tag="sd2")
                    nc.scalar.activation(sd2[:], var2[:], AF.Sqrt, bias=epsc[:])
                    r2 = stcol.tile([128, 1], f32, tag="r2")
                    nc.vector.reciprocal(r2[:], sd2[:])
                    c1 = stcol.tile([128, 1], f32, tag="c1")
                    nc.vector.tensor_mul(c1[:], rc, r2[:])
                    c0 = stcol.tile([128, 1], f32, tag="c0")
                    nc.vector.tensor_scalar(c0[:], mu2[:], r2[:], -1.0,
                                            ALU.mult, ALU.mult)

                    for dc in range(ND2):
                        fnat = fop.tile([128, 512], f32)
                        nc.vector.tensor_scalar(fnat[:], relu2[:, dc * 512:(dc + 1) * 512],
                                                c1[:], c0[:], ALU.mult, ALU.add)
                        nc.vector.tensor_mul(fnat[:], fnat[:],
                                             g2bc[:, dc * 512:(dc + 1) * 512])
                        fnr = fop.tile([128, 512], f32r, tag="fnr")
                        nc.vector.tensor_add(fnr[:], fnat[:],
                                             be2bc[:, dc * 512:(dc + 1) * 512])
                        nc.sync.dma_start(
                            f_d[s0 + sc * 128:s0 + (sc + 1) * 128,
                                dc * 512:(dc + 1) * 512], fnr[:])
                        for q in range(4):
                            ptr = psT.tile([128, 128], f32r, tag="pt")
                            nc.tensor.transpose(ptr[:], fnr[:, q * 128:(q + 1) * 128],
                                                eyer[:])
                            nc.scalar.copy(
                                fTacc[:, dc * 4 + q, sc * 128:(sc + 1) * 128],
                                ptr[:])
                nc.sync.dma_start(
                    fT_d[:, s0:s0 + SB].rearrange("(k p) s -> p k s", p=128),
                    fTacc[:])

        # ---------------- Phase 2a: scores, softmax, E, attn ----------------
        zic = None
        with tc.tile_pool(name="ftr", bufs=1) as ftr, \
             tc.tile_pool(name="ep", bufs=2) as ep, \
             tc.tile_pool(name="ap", bufs=2) as ap_, \
             tc.tile_pool(name="zp", bufs=1) as zp, \
             tc.tile_pool(name="psR", bufs=2, space="PSUM") as psR:
            fTres = ftr.tile([128, KH2, S], f32r)
            nc.sync.dma_start(fTres[:], fT_d.rearrange("(k p) t -> p k t", p=128))
            ziall = zp.tile([128, MT], f32)
            for a in range(MT):
                praw = psR.tile([128, S], f32)
                for n in range(NT):
                    for dt in range(KH2):
                        mmr(praw[:, n * 512:(n + 1) * 512],
                            fTres[:, dt, a * 128:(a + 1) * 128],
                            fTres[:, dt, n * 512:(n + 1) * 512],
                            start=(dt == 0), stop=(dt == KH2 - 1))
                et = ep.tile([128, S], f32r)
                zcol = zp.tile([128, 1], f32, tag="zcol")
                nc.scalar.activation(et[:], praw[:], AF.Exp, scale=float(SCALE),
                                     accum_out=zcol[:])
                nc.vector.reciprocal(ziall[:, a:a + 1], zcol[:])
                at = ap_.tile([128, S], f32)
                nc.vector.tensor_scalar_mul(at[:], et[:], ziall[:, a:a + 1])
                nc.sync.dma_start(attn_d[a * 128:(a + 1) * 128, :], at[:])
                nc.sync.dma_start(e_d[a * 128:(a + 1) * 128, :], et[:])
            nc.sync.dma_start(zi_d.rearrange("(a p) -> p a", p=128), ziall[:])

        # ---------------- Phase 2b: attended, head, epilogue ----------------
        with tc.tile_pool(name="fr", bufs=1) as fr, \
             tc.tile_pool(name="esp", bufs=6) as esp, \
             tc.tile_pool(name="atp", bufs=2) as atp, \
             tc.tile_pool(name="lgp", bufs=1) as lgp, \
             tc.tile_pool(name="lsp", bufs=1) as lsp, \
             tc.tile_pool(name="psAT", bufs=8, space="PSUM") as psAT:
            fres = fr.tile([128, MT, H2], f32r)
            nc.sync.dma_start(fres[:], f_d.rearrange("(t p) d -> p t d", p=128))
            zbc = lgp.tile([C, S], f32)
            nc.sync.dma_start(
                zbc[:], zi_d.rearrange("(o s) -> o s", o=1).to_broadcast((C, S)))
            logT = lgp.tile([C, S], f32)

            for n in range(NT):
                attT = atp.tile([128, KH2, 512], f32r)
                pms = [psAT.tile([128, 512], f32, tag="acc", name=f"pm{m}")
                       for m in range(KH2)]
                for tt in range(MT):
                    et = esp.tile([128, 512], f32r, tag="et")
                    nc.sync.dma_start(
                        et[:], e_d[tt * 128:(tt + 1) * 128,
                                   n * 512:(n + 1) * 512])
                    for m in range(KH2):
                        mmr(pms[m][:], fres[:, tt, m * 128:(m + 1) * 128],
                            et[:], start=(tt == 0), stop=(tt == MT - 1))
                for m in range(KH2):
                    nc.scalar.copy(attT[:, m, :], pms[m][:])
                ph = psAT.tile([128, 512], f32, tag="acc")
                for k in range(KH2):
                    mmr(ph[:], whsb[:, k, :], attT[:, k, :],
                        start=(k == 0), stop=(k == KH2 - 1))
                nc.scalar.copy(logT[:, n * 512:(n + 1) * 512], ph[0:C, :])

            # epilogue on [64, 2048]
            nc.vector.tensor_mul(logT[:], logT[:], zbc[:])
            sigl = lgp.tile([C, S], f32)
            nc.scalar.activation(sigl[:], logT[:], AF.Sigmoid, bias=bhc[:])
            sigp = lgp.tile([C, 1], f32)
            nc.scalar.activation(sigp[:], prc[:], AF.Sigmoid)
            andv = lgp.tile([C, S], f32)
            nc.vector.tensor_scalar(andv[:], sigl[:], sigp[:], agc[:],
                                    ALU.min, ALU.mult)
            orv = lgp.tile([C, S], f32)
            nc.vector.tensor_scalar(orv[:], sigl[:], sigp[:], ogc[:],
                                    ALU.max, ALU.mult)
            nc.vector.tensor_add(andv[:], andv[:], orv[:])
            nc.scalar.activation(logT[:], andv[:], AF.Ln, bias=e10c[:C], scale=0.5)

            lst = lsp.tile([128, MT, C], f32)
            for q in range(MT):
                pl = psAT.tile([128, C], f32, tag="acc")
                nc.tensor.transpose(pl[:], logT[:, q * 128:(q + 1) * 128],
                                    eye[:C, :C])
                nc.scalar.copy(lst[:, q, :], pl[:])
            nc.sync.dma_start(logit_d.rearrange("(q p) c -> p q c", p=128), lst[:])

        cp.release()

    nc.compile()
    return nc


def _get_nc():
    if "nc" not in _BUILT:
        _BUILT["nc"] = _build()
    return _BUILT["nc"]


def kernel(**inputs):
    from concourse.bass_utils import run_bass_kernel_spmd

    nc = _get_nc()
    emb = np.ascontiguousarray(np.asarray(inputs["embeddings"], dtype=np.float32))
    prior = np.ascontiguousarray(np.asarray(inputs["prior_evidence"], dtype=np.float32))
    common = {
        "w1": np.asarray(inputs["W1"], np.float32),
        "b1": np.asarray(inputs["b1"], np.float32),
        "g1": np.asarray(inputs["g1"], np.float32),
        "be1": np.asarray(inputs["beta1"], np.float32),
        "w2": np.asarray(inputs["W2"], np.float32),
        "b2": np.asarray(inputs["b2"], np.float32),
        "g2": np.asarray(inputs["g2"], np.float32),
        "be2": np.asarray(inputs["beta2"], np.float32),
        "wh": np.pad(np.asarray(inputs["Wh"], np.float32), ((0, 0), (0, 64))),
        "bh": np.asarray(inputs["bh"], np.float32),
        "ag": np.asarray(inputs["and_gate"], np.float32),
        "og": np.asarray(inputs["or_gate"], np.float32),
        "eye": np.eye(128, dtype=np.float32),
        "eyer": np.eye(128, dtype=np.float32),
        "ones": np.ones((128, 128), np.float32),
    }
    common = {k: np.ascontiguousarray(v) for k, v in common.items()}
    in_maps = [dict(common, x=emb[b], prior=prior[b]) for b in range(B)]
    res = run_bass_kernel_spmd(nc, in_maps, list(range(B)))
    logits = np.stack([res.results[b]["logits_out"] for b in range(B)])
    attn = np.stack([res.results[b]["attn_out"] for b in range(B)])
    return (logits, attn)


if __name__ == "__main__":
    _get_nc()
    print("build+compile OK")
